# revision 82
# baseline (speedup 1.0000x reference)
"""Trainium2 Bass kernel for sparse attention with relation bias.

Computes, for inputs (B=4, N=512, C=128, H=8, HS=16):
  qkv = joint @ W_qkv^T -> q,k,v
  attn = softmax((q k^T + rel @ W_r^T) * conn * HS^-0.5)
  out  = (attn @ v) @ W_proj^T + b_proj

Sharding: 8 cores, core i handles batch b=i//2 and n-row half i%2 (256 rows).
No collectives — each core computes its own output rows; host gathers.

Layout: packed attention rows. For each group of G=16 n-rows, logits live in
ONE [128, 512] tile with partition p = j*8 + h (j = n-row within group,
h = head). rel is pre-transposed and pre-cast to fp8-e4m3 on the host
([C, rows] layout) so the kernel streams it with plain HWDGE DMA straight
into the bias matmuls.

This revision is tuned so every engine's steady-state per-group cost sits
under the per-group DMA time (~3.4us for 1MB rel + 16KB conn), making the
kernel DMA-bound (the rel stream runs at ~320 GB/s, ~90% of the per-core
HBM limit). Measured levers vs the 95us baseline (~78us now):
 - ALL throttled DMA issues live on the sync engine's queue; putting any on
   the ACT ring stalls ACT's in-order compute queue behind DMA waits
 - every pool whose buffer-wait can appear in the sync queue is at least as
   deep as the rel prefetch horizon (connp=8), and the out-store issues are
   deferred past the loop — an issue instruction that waits mid-queue
   throttles every rel prefetch behind it
 - per-group out-projection + store replaced by one projection per 128-row
   half (removes ~1us/group of ACT time: PSUM evac + DMA issue)
 - rel streamed as 0.5MB half-group DMAs (host permutes each group's rows
   round-major so matmul rounds 0-1 need only half A) — finer completion
   granularity keeps the pipeline smooth; 1MB/2MB chunks measured slower
 - stage order oldest-first per iteration (tail g-2 | softmax g-1 | head g)
   so no engine's in-order queue blocks old ready work behind new
   DMA-dependent work; DMA-independent matmuls (conn select, J) precede the
   rel matmuls in the PE queue
 - qpack build on the otherwise-idle GPSIMD; conn evac alternates DVE/ACT
 - weights+jointT concatenated into ONE DMA on the scalar ring, parallel
   with the first rel load
Failed experiments (all measured slower): 2MB rel chunks, 0.25MB chunks,
4-stage pipeline, tail pairing (2 groups/tail), host-replicated conn,
early prefetch of the last groups on the scalar ring.
"""

import sys

sys.path.insert(0, "/opt/trn_rl_repo")

import numpy as np
import ml_dtypes

import concourse.bass as bass
import concourse.tile as tile
from concourse import bacc, mybir
from concourse.masks import make_identity
from contextlib import ExitStack

F32 = mybir.dt.float32
BF16 = mybir.dt.bfloat16
FP8 = mybir.dt.float8e4

# Problem constants (hardcoded per spec)
B, N, C, H = 4, 512, 128, 8
HS = C // H  # 16
SCALE = float(HS) ** -0.5
NCORES = 8
P = 128  # partitions
MC = N // P  # m-chunks per row = 4
G = 16  # n-rows per group


def build_graph(NH):
    """Build the SPMD single-core graph. NH = n-rows per core."""
    NG = NH // G  # groups

    nc = bacc.Bacc("TRN2", target_bir_lowering=False, debug=False)
    # bigw columns: wqkvT (3C) | wrT (H) | wprojT (C) | jointT (N) | jointTq (NH)
    BW = 3 * C + H + C + N + NH
    rel_d = nc.declare_dram_parameter("relT", [C, NH * N], FP8, isOutput=False)
    conn_d = nc.declare_dram_parameter("conn", [NH, N], BF16, isOutput=False)
    bigw_d = nc.declare_dram_parameter("bigw", [C, BW], BF16, isOutput=False)
    bp_d = nc.declare_dram_parameter("bproj", [1, C], BF16, isOutput=False)
    out_d = nc.declare_dram_parameter("out", [NH, C], F32, isOutput=True)

    with tile.TileContext(nc) as tc, ExitStack() as ctx:
        singles = ctx.enter_context(tc.tile_pool(name="singles", bufs=1))
        relp = ctx.enter_context(tc.tile_pool(name="relp", bufs=16))
        # conn issues share the sync ring with the rel stream: the conn pool
        # must be as deep as the rel prefetch horizon or its buffer-wait
        # stalls the in-order sync queue and throttles rel prefetch.
        connp = ctx.enter_context(tc.tile_pool(name="connp", bufs=8))
        connep = ctx.enter_context(tc.tile_pool(name="connep", bufs=3))
        qpackp = ctx.enter_context(tc.tile_pool(name="qpackp", bufs=2))
        logitp = ctx.enter_context(tc.tile_pool(name="logitp", bufs=3))
        attnep = ctx.enter_context(tc.tile_pool(name="attnep", bufs=3))
        attnwp = ctx.enter_context(tc.tile_pool(name="attnwp", bufs=3))
        aTp = ctx.enter_context(tc.tile_pool(name="aTp", bufs=2))
        xsbp = ctx.enter_context(tc.tile_pool(name="xsbp", bufs=2))
        smallp = ctx.enter_context(tc.tile_pool(name="smallp", bufs=8))
        outp = ctx.enter_context(tc.tile_pool(name="outp", bufs=2))

        # PSUM: 8 banks — attn 4, conn 1, tp 1, x 1, proj 1
        ps_attn = ctx.enter_context(tc.tile_pool(name="ps_attn", bufs=4, space="PSUM"))
        ps_conn = ctx.enter_context(tc.tile_pool(name="ps_conn", bufs=1, space="PSUM"))
        ps_tp = ctx.enter_context(tc.tile_pool(name="ps_tp", bufs=1, space="PSUM"))
        ps_x = ctx.enter_context(tc.tile_pool(name="ps_x", bufs=1, space="PSUM"))
        ps_o = ctx.enter_context(tc.tile_pool(name="ps_o", bufs=1, space="PSUM"))

        # ---- first rel DMA goes out on the sync ring before anything else ----
        # Each group is fetched as TWO 0.5MB halves; the host permutes rows so
        # half A holds rounds 0-1 (j%4 in {0,1}) and half B rounds 2-3 —
        # finer completion granularity, smoother pipeline.
        rel_tiles = {}

        def rel_fetch(g):
            a = relp.tile([P, (G // 2) * N], FP8, tag="rel", name="relga")
            nc.sync.dma_start(
                out=a, in_=rel_d[:, g * G * N : (g * G + G // 2) * N]
            )
            b = relp.tile([P, (G // 2) * N], FP8, tag="rel", name="relgb")
            nc.sync.dma_start(
                out=b, in_=rel_d[:, (g * G + G // 2) * N : (g + 1) * G * N]
            )
            rel_tiles[g] = (a, b)

        rel_fetch(0)

        # ---- constants / weights (scalar ring — runs parallel to rel[0]) ----
        # One concatenated DMA instead of six: the prep matmuls' inputs all
        # land ~3us earlier.
        bigw = singles.tile([P, BW], BF16)
        nc.scalar.dma_start(out=bigw, in_=bigw_d[:, :])
        wqkvT = bigw[:, : 3 * C]
        wrT = bigw[:, 3 * C : 3 * C + H]
        wpT = bigw[:, 3 * C + H : 3 * C + H + C]
        jT = bigw[:, 3 * C + H + C : 3 * C + H + C + N]
        jTq = bigw[:, 3 * C + H + C + N :]
        bp = singles.tile([1, C], BF16)
        nc.scalar.dma_start(out=bp, in_=bp_d[:, :])

        ident = singles.tile([P, P], BF16)
        make_identity(nc, ident)
        ones128 = singles.tile([1, P], BF16)
        nc.vector.memset(ones128, 1.0)

        # Mask[c, j*8+h] = 1.0 iff c//16 == h
        mask = singles.tile([P, P], BF16)
        nc.gpsimd.memset(mask, 1.0)
        nc.gpsimd.affine_select(
            out=mask, in_=mask, compare_op=mybir.AluOpType.is_ge, fill=0.0,
            base=0, pattern=[[0, G], [-HS, H]], channel_multiplier=1,
        )
        nc.gpsimd.affine_select(
            out=mask, in_=mask, compare_op=mybir.AluOpType.is_ge, fill=0.0,
            base=HS - 1, pattern=[[0, G], [HS, H]], channel_multiplier=-1,
        )

        # sel16[j, j*8+h] = 1.0 — broadcasts conn rows x8 via the PE.
        sel16 = singles.tile([G, P], BF16)
        nc.gpsimd.memset(sel16, 1.0)
        nc.gpsimd.affine_select(
            out=sel16, in_=sel16, compare_op=mybir.AluOpType.is_ge, fill=0.0,
            base=0, pattern=[[1, P]], channel_multiplier=-H,
        )
        nc.gpsimd.affine_select(
            out=sel16, in_=sel16, compare_op=mybir.AluOpType.is_ge, fill=0.0,
            base=H - 1, pattern=[[-1, P]], channel_multiplier=H,
        )

        # wrj4[c, r, (j%4)*8+h] = W_r[h, c] for r == j%4; zero elsewhere.
        wrj4 = singles.tile([P, 4, 32], BF16)
        nc.vector.memset(wrj4, 0.0)
        for r in range(4):
            nc.vector.tensor_copy(wrj4[:, r, r * H : (r + 1) * H], wrT)

        # ---- prep: natural-layout qkv projections ----
        kT = singles.tile([P, N], BF16)
        vnat = singles.tile([P, MC, P], BF16)
        qTq = singles.tile([P, NH], BF16)

        pk = ps_attn.tile([P, N], F32, tag="attn")
        nc.tensor.matmul(pk, lhsT=wqkvT[:, C : 2 * C], rhs=jT, start=True, stop=True)
        nc.vector.tensor_copy(kT, pk)
        pq = ps_attn.tile([P, N], F32, tag="attn")
        nc.tensor.matmul(pq[:, :NH], lhsT=wqkvT[:, :C], rhs=jTq, start=True, stop=True)
        nc.vector.tensor_copy(qTq, pq[:, :NH])
        for t in range(MC):
            pv = ps_attn.tile([P, N], F32, tag="attn")
            nc.tensor.matmul(
                pv[:, :P], lhsT=jT[:, t * P : (t + 1) * P],
                rhs=wqkvT[:, 2 * C :], start=True, stop=True,
            )
            nc.vector.tensor_copy(vnat[:, t, :], pv[:, :P])

        # per-half output accumulator [c_in, half, j] and its projection
        xsh = singles.tile([P, NH // P, P], BF16)

        head_out = {}
        sm_out = {}
        pending_stores = []

        def phase_head(g):
            if g not in rel_tiles:
                rel_fetch(g)
            relA, relB = rel_tiles[g]

            # conn rows for the group + x8 replication via selector matmul
            # (PE). The DMA-independent matmuls (conn select, J) go to the PE
            # queue BEFORE the rel matmuls so a late rel DMA can't block them.
            conn_g = connp.tile([G, N], BF16)
            nc.sync.dma_start(out=conn_g, in_=conn_d[g * G : (g + 1) * G, :])
            conn_ps = ps_conn.tile([P, N], F32, tag="conn")
            nc.tensor.matmul(conn_ps, lhsT=sel16, rhs=conn_g, start=True, stop=True)
            conn_e = connep.tile([P, N], BF16)
            if g % 2 == 0:
                nc.vector.tensor_copy(conn_e, conn_ps)
            else:
                nc.scalar.copy(conn_e, conn_ps)

            # Qpacked[c, j*8+h] = qTq[c, g*16+j] * Mask[c, j*8+h]  (GPSIMD)
            qpack = qpackp.tile([P, P], BF16)
            qsrc = qTq[:, g * G : (g + 1) * G]
            qrep = bass.AP(
                tensor=qsrc.tensor,
                offset=qsrc.offset,
                ap=[qsrc.ap[0], qsrc.ap[1], [0, H]],
            )
            nc.gpsimd.tensor_mul(
                qpack.rearrange("p (j h) -> p j h", h=H),
                qrep,
                mask.rearrange("p (j h) -> p j h", h=H),
            )

            # logits: J + R accumulated into one PSUM bank
            Pattn = ps_attn.tile([P, N], F32, tag="attn")
            nc.tensor.matmul(Pattn, lhsT=qpack, rhs=kT, start=True, stop=False)
            for r in range(4):
                half_t = relA if r < 2 else relB
                for q in range(4):
                    ph = (r % 2) * 4 + q  # block index within the half tile
                    nc.tensor.matmul(
                        Pattn[q * 32 : (q + 1) * 32, :],
                        lhsT=wrj4[:, r, :],
                        rhs=half_t[:, ph * N : (ph + 1) * N],
                        start=False,
                        stop=(r == 3),
                        tile_position=(0, q * 32),
                        skip_group_check=True,
                    )
            head_out[g] = (Pattn, conn_e)

        def phase_softmax(g):
            Pattn, conn_e = head_out.pop(g)
            logits = logitp.tile([P, N], BF16)
            nc.vector.tensor_mul(logits, Pattn, conn_e)
            attn_e = attnep.tile([P, N], BF16)
            sums = smallp.tile([P, 1], F32)
            nc.scalar.activation(
                out=attn_e, in_=logits,
                func=mybir.ActivationFunctionType.Exp,
                scale=SCALE, accum_out=sums,
            )
            recip = smallp.tile([P, 1], F32)
            nc.vector.reciprocal(recip, sums)
            attn_w = attnwp.tile([P, N], BF16)
            nc.vector.tensor_scalar_mul(attn_w, attn_e, recip)
            sm_out[g] = attn_w

        def phase_tail(g):
            attn_w = sm_out.pop(g)
            # aT[m_local, (chunk, j*8+h)] via transpose-mode; evac on ACT.
            PT = ps_tp.tile([P, N], BF16, tag="tp")
            aT = aTp.tile([P, N], BF16)
            for half in range(2):
                for c in (2 * half, 2 * half + 1):
                    nc.tensor.transpose(
                        PT[:, c * P : (c + 1) * P], attn_w[:, c * P : (c + 1) * P], ident
                    )
                nc.scalar.copy(
                    aT[:, half * 2 * P : (half + 1) * 2 * P],
                    PT[:, half * 2 * P : (half + 1) * 2 * P],
                )

            # x' = attn @ v (with cross-head garbage), masked+reduced to xs2
            PX = ps_x.tile([P, P], F32, tag="px")
            for c in range(MC):
                nc.tensor.matmul(
                    PX, lhsT=vnat[:, c, :], rhs=aT[:, c * P : (c + 1) * P],
                    start=(c == 0), stop=(c == MC - 1),
                )
            xsb = xsbp.tile([P, P], BF16)
            nc.vector.tensor_mul(xsb, PX, mask)
            xs2 = smallp.tile([P, G], F32)
            nc.vector.reduce_sum(
                xs2, xsb.rearrange("p (j h) -> p j h", h=H), axis=mybir.AxisListType.X
            )
            half, slot = divmod(g, P // G)
            nc.vector.tensor_copy(xsh[:, half, slot * G : (slot + 1) * G], xs2)

            if slot == P // G - 1:
                # project this 128-row half; the store ISSUE is deferred to
                # after the loop so its evac-wait never stalls the sync
                # queue's rel prefetch mid-stream.
                PO = ps_o.tile([P, C], F32)
                nc.tensor.matmul(PO, lhsT=xsh[:, half, :], rhs=wpT, start=True, stop=False)
                nc.tensor.matmul(PO, lhsT=ones128, rhs=bp, start=False, stop=True)
                out_sb = outp.tile([P, C], F32)
                nc.scalar.copy(out_sb, PO)
                pending_stores.append((half, out_sb))

        # Stage order oldest-first so each engine's in-order queue never has
        # newer-stage work (which may wait on a fresh DMA) blocking older-stage
        # work: tail(g-2) | softmax(g-1) | head(g).
        for it in range(NG + 2):
            if it >= 2:
                phase_tail(it - 2)
            if 1 <= it <= NG:
                phase_softmax(it - 1)
            if it < NG:
                phase_head(it)

        for half, out_sb in pending_stores:
            nc.sync.dma_start(out=out_d[half * P : (half + 1) * P, :], in_=out_sb)

    return nc


_GRAPH_CACHE = {}


def _get_graph(NH):
    if NH not in _GRAPH_CACHE:
        nc = build_graph(NH)
        nc.finalize()
        _GRAPH_CACHE[NH] = nc
    return _GRAPH_CACHE[NH]


def _bf16(x):
    return np.ascontiguousarray(x.astype(ml_dtypes.bfloat16))


def make_in_maps(joint_feature, relation_feature, conn_feature, W_qkv, W_r, W_proj, b_proj):
    """Shard full inputs into 8 per-core input maps (layout/dtype prep only)."""
    NH = N // 2
    wqkvT = W_qkv.T  # [C_in, 3C_out]: q | k | v column sections
    bp = _bf16(b_proj[None, :])
    in_maps = []
    for core in range(NCORES):
        b = core // 2
        half = core % 2
        n0 = half * NH
        jT = joint_feature[b].T
        jTq = joint_feature[b, n0 : n0 + NH].T
        bigw = _bf16(np.concatenate([wqkvT, W_r.T, W_proj.T, jT, jTq], axis=1))
        # permute rows within each group of 16 so DMA-half A holds rounds
        # 0-1 (j = q*4+r with r<2) and half B rounds 2-3: position p = r*4+q
        perm = np.array([(p % 4) * 4 + p // 4 for p in range(G)])
        idx = (np.arange(NH).reshape(-1, G)[:, perm]).ravel()
        relT = np.ascontiguousarray(
            relation_feature[b, n0 : n0 + NH][idx]
            .reshape(NH * N, C)
            .T.astype(ml_dtypes.float8_e4m3)
        )
        conn = _bf16(conn_feature[b, n0 : n0 + NH])
        in_maps.append(
            {
                "relT": relT,
                "conn": conn,
                "bigw": bigw,
                "bproj": bp,
            }
        )
    return in_maps


def kernel(joint_feature, relation_feature, conn_feature, W_qkv, W_r, W_proj, b_proj):
    from concourse.bass_utils import run_bass_kernel_spmd

    NH = N // 2
    nc = _get_graph(NH)
    in_maps = make_in_maps(
        joint_feature, relation_feature, conn_feature, W_qkv, W_r, W_proj, b_proj
    )
    res = run_bass_kernel_spmd(nc, in_maps, core_ids=list(range(NCORES)))
    out = np.zeros((B, N, C), dtype=np.float32)
    for core in range(NCORES):
        b = core // 2
        half = core % 2
        n0 = half * NH
        out[b, n0 : n0 + NH] = res.results[core]["out"]
    return out


# revision 83
# speedup vs baseline: 1.0738x; 1.0738x over previous
"""Trainium2 Bass kernel for sparse attention with relation bias.

Computes, for inputs (B=4, N=512, C=128, H=8, HS=16):
  qkv = joint @ W_qkv^T -> q,k,v
  attn = softmax((q k^T + rel @ W_r^T) * conn * HS^-0.5)
  out  = (attn @ v) @ W_proj^T + b_proj

Sharding: 8 cores, core i handles batch b=i//2 and n-row half i%2 (256 rows).
No collectives — each core computes its own output rows; host gathers.

Layout: packed attention rows. For each group of G=16 n-rows, logits live in
ONE [128, 512] tile with partition p = j*8 + h (j = n-row within group,
h = head). rel is pre-transposed and pre-cast to fp8-e4m3 on the host
([C, rows] layout) so the kernel streams it with plain HWDGE DMA straight
into the bias matmuls.

This revision is tuned so every engine's steady-state per-group cost sits
under the per-group DMA time (~3.4us for 1MB rel + 16KB conn), making the
kernel DMA-bound (the rel stream runs at ~320 GB/s, ~90% of the per-core
HBM limit). Measured levers vs the 95us baseline (~78us now):
 - ALL throttled DMA issues live on the sync engine's queue; putting any on
   the ACT ring stalls ACT's in-order compute queue behind DMA waits
 - every pool whose buffer-wait can appear in the sync queue is at least as
   deep as the rel prefetch horizon (connp=8), and the out-store issues are
   deferred past the loop — an issue instruction that waits mid-queue
   throttles every rel prefetch behind it
 - per-group out-projection + store replaced by one projection per 128-row
   half (removes ~1us/group of ACT time: PSUM evac + DMA issue)
 - rel streamed as 0.5MB half-group DMAs (host permutes each group's rows
   round-major so matmul rounds 0-1 need only half A) — finer completion
   granularity keeps the pipeline smooth; 1MB/2MB chunks measured slower
 - stage order oldest-first per iteration (tail g-2 | softmax g-1 | head g)
   so no engine's in-order queue blocks old ready work behind new
   DMA-dependent work; DMA-independent matmuls (conn select, J) precede the
   rel matmuls in the PE queue
 - qpack build on the otherwise-idle GPSIMD; conn evac alternates DVE/ACT
 - weights+jointT concatenated into ONE DMA on the scalar ring, parallel
   with the first rel load
Failed experiments (all measured slower): 2MB rel chunks, 0.25MB chunks,
4-stage pipeline, tail pairing (2 groups/tail), host-replicated conn,
early prefetch of the last groups on the scalar ring.
"""

import sys

sys.path.insert(0, "/opt/trn_rl_repo")

import numpy as np
import ml_dtypes

import concourse.bass as bass
import concourse.tile as tile
from concourse import bacc, mybir
from concourse.masks import make_identity
from contextlib import ExitStack

F32 = mybir.dt.float32
BF16 = mybir.dt.bfloat16
FP8 = mybir.dt.float8e4

# Problem constants (hardcoded per spec)
B, N, C, H = 4, 512, 128, 8
HS = C // H  # 16
SCALE = float(HS) ** -0.5
NCORES = 8
P = 128  # partitions
MC = N // P  # m-chunks per row = 4
G = 16  # n-rows per group


def build_graph(NH):
    """Build the SPMD single-core graph. NH = n-rows per core."""
    NG = NH // G  # groups

    nc = bacc.Bacc("TRN2", target_bir_lowering=False, debug=False)
    # bigw columns: wqkvT (3C) | wrT (H) | wprojT (C) | jointT (N) | jointTq (NH)
    BW = 3 * C + H + C + N + NH
    rel_d = nc.declare_dram_parameter("relT", [C, NH * N], FP8, isOutput=False)
    conn_d = nc.declare_dram_parameter("conn", [NH, N], BF16, isOutput=False)
    bigw_d = nc.declare_dram_parameter("bigw", [C, BW], BF16, isOutput=False)
    bp_d = nc.declare_dram_parameter("bproj", [1, C], BF16, isOutput=False)
    out_d = nc.declare_dram_parameter("out", [NH, C], F32, isOutput=True)

    with tile.TileContext(nc) as tc, ExitStack() as ctx:
        singles = ctx.enter_context(tc.tile_pool(name="singles", bufs=1))
        relp = ctx.enter_context(tc.tile_pool(name="relp", bufs=16))
        # conn issues share the sync ring with the rel stream: the conn pool
        # must be as deep as the rel prefetch horizon or its buffer-wait
        # stalls the in-order sync queue and throttles rel prefetch.
        connp = ctx.enter_context(tc.tile_pool(name="connp", bufs=8))
        connep = ctx.enter_context(tc.tile_pool(name="connep", bufs=3))
        qpackp = ctx.enter_context(tc.tile_pool(name="qpackp", bufs=3))
        logitp = ctx.enter_context(tc.tile_pool(name="logitp", bufs=4))
        attnep = ctx.enter_context(tc.tile_pool(name="attnep", bufs=4))
        attnwp = ctx.enter_context(tc.tile_pool(name="attnwp", bufs=4))
        aTp = ctx.enter_context(tc.tile_pool(name="aTp", bufs=3))
        xsbp = ctx.enter_context(tc.tile_pool(name="xsbp", bufs=3))
        smallp = ctx.enter_context(tc.tile_pool(name="smallp", bufs=12))
        outp = ctx.enter_context(tc.tile_pool(name="outp", bufs=2))

        # PSUM: 8 banks — attn 4, conn 1, tp 1, x 1, proj 1
        ps_attn = ctx.enter_context(tc.tile_pool(name="ps_attn", bufs=4, space="PSUM"))
        ps_conn = ctx.enter_context(tc.tile_pool(name="ps_conn", bufs=1, space="PSUM"))
        ps_tp = ctx.enter_context(tc.tile_pool(name="ps_tp", bufs=1, space="PSUM"))
        ps_x = ctx.enter_context(tc.tile_pool(name="ps_x", bufs=1, space="PSUM"))
        ps_o = ctx.enter_context(tc.tile_pool(name="ps_o", bufs=1, space="PSUM"))

        # ---- first rel DMA goes out on the sync ring before anything else ----
        # Each group is fetched as TWO 0.5MB halves; the host permutes rows so
        # half A holds rounds 0-1 (j%4 in {0,1}) and half B rounds 2-3 —
        # finer completion granularity, smoother pipeline.
        rel_tiles = {}

        def rel_fetch(g):
            a = relp.tile([P, (G // 2) * N], FP8, tag="rel", name="relga")
            nc.sync.dma_start(
                out=a, in_=rel_d[:, g * G * N : (g * G + G // 2) * N]
            )
            b = relp.tile([P, (G // 2) * N], FP8, tag="rel", name="relgb")
            nc.sync.dma_start(
                out=b, in_=rel_d[:, (g * G + G // 2) * N : (g + 1) * G * N]
            )
            rel_tiles[g] = (a, b)

        rel_fetch(0)

        # ---- constants / weights (scalar ring — runs parallel to rel[0]) ----
        # One concatenated DMA instead of six: the prep matmuls' inputs all
        # land ~3us earlier.
        bigw = singles.tile([P, BW], BF16)
        nc.scalar.dma_start(out=bigw, in_=bigw_d[:, :])
        wqkvT = bigw[:, : 3 * C]
        wrT = bigw[:, 3 * C : 3 * C + H]
        wpT = bigw[:, 3 * C + H : 3 * C + H + C]
        jT = bigw[:, 3 * C + H + C : 3 * C + H + C + N]
        jTq = bigw[:, 3 * C + H + C + N :]
        bp = singles.tile([1, C], BF16)
        nc.scalar.dma_start(out=bp, in_=bp_d[:, :])

        ident = singles.tile([P, P], BF16)
        make_identity(nc, ident)
        ones128 = singles.tile([1, P], BF16)
        nc.vector.memset(ones128, 1.0)

        # Mask[c, j*8+h] = 1.0 iff c//16 == h
        mask = singles.tile([P, P], BF16)
        nc.gpsimd.memset(mask, 1.0)
        nc.gpsimd.affine_select(
            out=mask, in_=mask, compare_op=mybir.AluOpType.is_ge, fill=0.0,
            base=0, pattern=[[0, G], [-HS, H]], channel_multiplier=1,
        )
        nc.gpsimd.affine_select(
            out=mask, in_=mask, compare_op=mybir.AluOpType.is_ge, fill=0.0,
            base=HS - 1, pattern=[[0, G], [HS, H]], channel_multiplier=-1,
        )

        # sel16[j, j*8+h] = 1.0 — broadcasts conn rows x8 via the PE.
        sel16 = singles.tile([G, P], BF16)
        nc.gpsimd.memset(sel16, 1.0)
        nc.gpsimd.affine_select(
            out=sel16, in_=sel16, compare_op=mybir.AluOpType.is_ge, fill=0.0,
            base=0, pattern=[[1, P]], channel_multiplier=-H,
        )
        nc.gpsimd.affine_select(
            out=sel16, in_=sel16, compare_op=mybir.AluOpType.is_ge, fill=0.0,
            base=H - 1, pattern=[[-1, P]], channel_multiplier=H,
        )

        # wrj4[c, r, (j%4)*8+h] = W_r[h, c] for r == j%4; zero elsewhere.
        wrj4 = singles.tile([P, 4, 32], BF16)
        nc.vector.memset(wrj4, 0.0)
        for r in range(4):
            nc.vector.tensor_copy(wrj4[:, r, r * H : (r + 1) * H], wrT)

        # ---- prep: natural-layout qkv projections ----
        kT = singles.tile([P, N], BF16)
        vnat = singles.tile([P, MC, P], BF16)
        qTq = singles.tile([P, NH], BF16)

        pk = ps_attn.tile([P, N], F32, tag="attn")
        nc.tensor.matmul(pk, lhsT=wqkvT[:, C : 2 * C], rhs=jT, start=True, stop=True)
        nc.vector.tensor_copy(kT, pk)
        pq = ps_attn.tile([P, N], F32, tag="attn")
        nc.tensor.matmul(pq[:, :NH], lhsT=wqkvT[:, :C], rhs=jTq, start=True, stop=True)
        nc.vector.tensor_copy(qTq, pq[:, :NH])
        for t in range(MC):
            pv = ps_attn.tile([P, N], F32, tag="attn")
            nc.tensor.matmul(
                pv[:, :P], lhsT=jT[:, t * P : (t + 1) * P],
                rhs=wqkvT[:, 2 * C :], start=True, stop=True,
            )
            nc.vector.tensor_copy(vnat[:, t, :], pv[:, :P])

        # per-half output accumulator [c_in, half, j] and its projection
        xsh = singles.tile([P, NH // P, P], BF16)

        head_out = {}
        sm_out = {}
        pending_stores = []

        def phase_head(g):
            if g not in rel_tiles:
                rel_fetch(g)
            relA, relB = rel_tiles[g]

            # conn rows for the group + x8 replication via selector matmul
            # (PE). The DMA-independent matmuls (conn select, J) go to the PE
            # queue BEFORE the rel matmuls so a late rel DMA can't block them.
            conn_g = connp.tile([G, N], BF16)
            nc.sync.dma_start(out=conn_g, in_=conn_d[g * G : (g + 1) * G, :])
            conn_ps = ps_conn.tile([P, N], F32, tag="conn")
            nc.tensor.matmul(conn_ps, lhsT=sel16, rhs=conn_g, start=True, stop=True)
            conn_e = connep.tile([P, N], BF16)
            if g % 2 == 0:
                nc.vector.tensor_copy(conn_e, conn_ps)
            else:
                nc.scalar.copy(conn_e, conn_ps)

            # Qpacked[c, j*8+h] = qTq[c, g*16+j] * Mask[c, j*8+h]  (GPSIMD)
            qpack = qpackp.tile([P, P], BF16)
            qsrc = qTq[:, g * G : (g + 1) * G]
            qrep = bass.AP(
                tensor=qsrc.tensor,
                offset=qsrc.offset,
                ap=[qsrc.ap[0], qsrc.ap[1], [0, H]],
            )
            nc.gpsimd.tensor_mul(
                qpack.rearrange("p (j h) -> p j h", h=H),
                qrep,
                mask.rearrange("p (j h) -> p j h", h=H),
            )

            # logits: J + R accumulated into one PSUM bank
            Pattn = ps_attn.tile([P, N], F32, tag="attn")
            nc.tensor.matmul(Pattn, lhsT=qpack, rhs=kT, start=True, stop=False)
            for r in range(4):
                half_t = relA if r < 2 else relB
                for q in range(4):
                    ph = (r % 2) * 4 + q  # block index within the half tile
                    nc.tensor.matmul(
                        Pattn[q * 32 : (q + 1) * 32, :],
                        lhsT=wrj4[:, r, :],
                        rhs=half_t[:, ph * N : (ph + 1) * N],
                        start=False,
                        stop=(r == 3),
                        tile_position=(0, q * 32),
                        skip_group_check=True,
                    )
            head_out[g] = (Pattn, conn_e)

        def phase_softmax(g):
            Pattn, conn_e = head_out.pop(g)
            logits = logitp.tile([P, N], BF16)
            nc.vector.tensor_mul(logits, Pattn, conn_e)
            attn_e = attnep.tile([P, N], BF16)
            sums = smallp.tile([P, 1], F32)
            nc.scalar.activation(
                out=attn_e, in_=logits,
                func=mybir.ActivationFunctionType.Exp,
                scale=SCALE, accum_out=sums,
            )
            recip = smallp.tile([P, 1], F32)
            nc.vector.reciprocal(recip, sums)
            attn_w = attnwp.tile([P, N], BF16)
            nc.vector.tensor_scalar_mul(attn_w, attn_e, recip)
            sm_out[g] = attn_w

        def phase_tail(g):
            attn_w = sm_out.pop(g)
            # aT[m_local, (chunk, j*8+h)] via transpose-mode; evac on ACT.
            PT = ps_tp.tile([P, N], BF16, tag="tp")
            aT = aTp.tile([P, N], BF16)
            for half in range(2):
                for c in (2 * half, 2 * half + 1):
                    nc.tensor.transpose(
                        PT[:, c * P : (c + 1) * P], attn_w[:, c * P : (c + 1) * P], ident
                    )
                nc.scalar.copy(
                    aT[:, half * 2 * P : (half + 1) * 2 * P],
                    PT[:, half * 2 * P : (half + 1) * 2 * P],
                )

            # x' = attn @ v (with cross-head garbage), masked+reduced to xs2
            PX = ps_x.tile([P, P], F32, tag="px")
            for c in range(MC):
                nc.tensor.matmul(
                    PX, lhsT=vnat[:, c, :], rhs=aT[:, c * P : (c + 1) * P],
                    start=(c == 0), stop=(c == MC - 1),
                )
            xsb = xsbp.tile([P, P], BF16)
            nc.vector.tensor_mul(xsb, PX, mask)
            xs2 = smallp.tile([P, G], F32)
            nc.vector.reduce_sum(
                xs2, xsb.rearrange("p (j h) -> p j h", h=H), axis=mybir.AxisListType.X
            )
            half, slot = divmod(g, P // G)
            nc.vector.tensor_copy(xsh[:, half, slot * G : (slot + 1) * G], xs2)

            if slot == P // G - 1:
                # project this 128-row half; the store ISSUE is deferred to
                # after the loop so its evac-wait never stalls the sync
                # queue's rel prefetch mid-stream.
                PO = ps_o.tile([P, C], F32)
                nc.tensor.matmul(PO, lhsT=xsh[:, half, :], rhs=wpT, start=True, stop=False)
                nc.tensor.matmul(PO, lhsT=ones128, rhs=bp, start=False, stop=True)
                out_sb = outp.tile([P, C], F32)
                nc.scalar.copy(out_sb, PO)
                pending_stores.append((half, out_sb))

        # Stage order oldest-first so each engine's in-order queue never has
        # newer-stage work (which may wait on a fresh DMA) blocking older-stage
        # work: tail(g-2) | softmax(g-1) | head(g).
        for it in range(NG + 2):
            if it >= 2:
                phase_tail(it - 2)
            if 1 <= it <= NG:
                phase_softmax(it - 1)
            if it < NG:
                phase_head(it)

        for half, out_sb in pending_stores:
            nc.sync.dma_start(out=out_d[half * P : (half + 1) * P, :], in_=out_sb)

    return nc


_GRAPH_CACHE = {}


def _get_graph(NH):
    if NH not in _GRAPH_CACHE:
        nc = build_graph(NH)
        nc.finalize()
        _GRAPH_CACHE[NH] = nc
    return _GRAPH_CACHE[NH]


def _bf16(x):
    return np.ascontiguousarray(x.astype(ml_dtypes.bfloat16))


def make_in_maps(joint_feature, relation_feature, conn_feature, W_qkv, W_r, W_proj, b_proj):
    """Shard full inputs into 8 per-core input maps (layout/dtype prep only)."""
    NH = N // 2
    wqkvT = W_qkv.T  # [C_in, 3C_out]: q | k | v column sections
    bp = _bf16(b_proj[None, :])
    in_maps = []
    for core in range(NCORES):
        b = core // 2
        half = core % 2
        n0 = half * NH
        jT = joint_feature[b].T
        jTq = joint_feature[b, n0 : n0 + NH].T
        bigw = _bf16(np.concatenate([wqkvT, W_r.T, W_proj.T, jT, jTq], axis=1))
        # permute rows within each group of 16 so DMA-half A holds rounds
        # 0-1 (j = q*4+r with r<2) and half B rounds 2-3: position p = r*4+q
        perm = np.array([(p % 4) * 4 + p // 4 for p in range(G)])
        idx = (np.arange(NH).reshape(-1, G)[:, perm]).ravel()
        relT = np.ascontiguousarray(
            relation_feature[b, n0 : n0 + NH][idx]
            .reshape(NH * N, C)
            .T.astype(ml_dtypes.float8_e4m3)
        )
        conn = _bf16(conn_feature[b, n0 : n0 + NH])
        in_maps.append(
            {
                "relT": relT,
                "conn": conn,
                "bigw": bigw,
                "bproj": bp,
            }
        )
    return in_maps


def kernel(joint_feature, relation_feature, conn_feature, W_qkv, W_r, W_proj, b_proj):
    from concourse.bass_utils import run_bass_kernel_spmd

    NH = N // 2
    nc = _get_graph(NH)
    in_maps = make_in_maps(
        joint_feature, relation_feature, conn_feature, W_qkv, W_r, W_proj, b_proj
    )
    res = run_bass_kernel_spmd(nc, in_maps, core_ids=list(range(NCORES)))
    out = np.zeros((B, N, C), dtype=np.float32)
    for core in range(NCORES):
        b = core // 2
        half = core % 2
        n0 = half * NH
        out[b, n0 : n0 + NH] = res.results[core]["out"]
    return out
